# revision 12
# baseline (speedup 1.0000x reference)
"""CCPL contrastive-loss kernel for Trainium2 (8 NeuronCores).

The loss touches only 256 sampled 3x3 neighborhoods of the 512 MiB feat
tensors, so the kernel gathers exactly those windows and never streams the
full tensors.  Work is data-parallel over batch: core b gets feat_q[b] and
feat_k[b], staged to HBM as a 3-row-interleaved pixel-major bf16 tensor
  T[h*W + w] = [f(h+r, w, c) for r in 0..2 for c in 0..127]   (768 B/row)
(q's 64 channels then k's 64 channels per pixel; rows h+1, h+2 duplicated
into neighboring staged rows).  With this layout one sample's whole 3x3
window = 3 CONSECUTIVE staged rows = one contiguous 2304 B run.

Device dataflow per core:
  1. TWO indirect (SWDGE) DMAs -- one offset per partition, the
     hardware-verified form -- gather 128 samples each: partition p of
     instruction i receives the full window of sample i*128+p as
     [3(dw), 3(r), 2(q/k), 64(ch)] bf16 in its free dim.  sample_ids
     become a [128, 2] int32 SBUF offset table (h*512+w).
  2. Everything else is per-partition elementwise: center subtract
     (bf16, DVE 2x), square to f32 (ACT), channel-norm reduce (DVE),
     sqrt(n + eps^2) via ACT bias (== sqrt(n)+eps to 1e-8 rel), and the
     normalized difference via the exact factorization
         |qh - kh| = rq * |dq - (rk*(sqrt(nq)+eps)) * dk|
     which saves one full-width multiply pass.  One q-k subtract runs on
     GPSIMD to offload DVE.
  3. Partials [128, 2, 3, 3] DMA out per block; host sums and divides.

The center slot produces exactly zero contribution (its window difference
is identically 0 and rho=1 there), so no masking is needed.
"""

import os
import sys
from contextlib import ExitStack

import ml_dtypes
import numpy as np

sys.path.insert(0, "/opt/trn_rl_repo")

import concourse.bass as bass
import concourse.tile as tile
from concourse import mybir
from concourse.bass_utils import run_bass_kernel_spmd


def _install_ntff_hook():
    """Provide antenv.axon_hooks when the agent image lacks it."""
    try:
        from antenv.axon_hooks import get_axon_ntff_profile_hook  # noqa: F401

        return
    except ImportError:
        pass
    import types

    hook = None
    try:
        from trn_agent_boot.trn_boot import _ntff_profile_via_ctypes

        so = "/opt/axon/libaxon_pjrt.so"
        if os.path.exists(so):
            hook = _ntff_profile_via_ctypes(so)
    except Exception:
        hook = None
    mod = types.ModuleType("antenv.axon_hooks")
    _state = {"hook": hook}
    mod.get_axon_ntff_profile_hook = lambda: _state["hook"]
    mod.set_axon_ntff_profile_hook = lambda h: _state.update(hook=h)
    import antenv

    sys.modules["antenv.axon_hooks"] = mod
    antenv.axon_hooks = mod


_install_ntff_hook()

B, C, H, W = 8, 64, 512, 512
NUM_S = 256
EPS = 1e-7
P = 128
HWPIX = H * W
NI = NUM_S // P     # 2 gather instructions, 128 samples each
FR = 3 * 2 * C      # 384 elements per staged row
N_CORES = 8

_cache: dict = {}
LAST_RESULTS = None  # BassKernelResults of the most recent run (for test.py)


def _split_multi_waits(nc):
    """Walrus build here embeds at most ONE sync wait per instruction."""
    from concourse import mybir as _mybir

    for f in nc.m.functions:
        for blk in f.blocks:
            insts = blk.instructions
            i = 0
            while i < len(insts):
                inst = insts[i]
                si = inst.sync_info
                if si is not None and si.on_wait and len(si.on_wait) > 1:
                    waits = list(si.on_wait)
                    si.on_wait = waits[-1:]
                    for j, w in enumerate(waits[:-1]):
                        nop = _mybir.InstNoOp(
                            name=nc.get_next_instruction_name(),
                            ins=[],
                            outs=[],
                            engine=inst.engine,
                            sync_info=_mybir.SyncInfo(on_wait=[w], on_update=[]),
                        )
                        insts.insert(i + j, nop)
                    i += len(waits) - 1
                i += 1


def _build():
    f32 = mybir.dt.float32
    bf16 = mybir.dt.bfloat16
    i32 = mybir.dt.int32
    TT = mybir.AluOpType
    nc = bass.Bass()
    fqk3 = nc.dram_tensor("fqk3", [HWPIX, FR], bf16, kind="ExternalInput")
    offs = nc.dram_tensor("offs", [P, NI], i32, kind="ExternalInput")
    out_su = nc.dram_tensor("out_su", [P, NI, 3, 3], bf16, kind="ExternalOutput")
    out_rr = nc.dram_tensor("out_rr", [P, NI, 3, 3, 2], f32, kind="ExternalOutput")

    with tile.TileContext(nc) as tc, ExitStack() as ctx:
        sb = ctx.enter_context(tc.tile_pool(name="sb", bufs=1))
        work = ctx.enter_context(tc.tile_pool(name="work", bufs=1))

        offt = sb.tile([P, NI], i32)
        nc.sync.dma_start(out=offt[:], in_=offs[:])
        eps2 = sb.tile([P, 1], f32)
        nc.vector.memset(eps2[:], EPS * EPS)

        X = []
        for i in range(NI):
            Xi = work.tile([P, 3, 3, 2, C], bf16, tag=f"x{i}")
            nc.gpsimd.indirect_dma_start(
                out=Xi[:].rearrange("p a b t c -> p (a b t c)"),
                out_offset=None,
                in_=fqk3[:],
                in_offset=bass.IndirectOffsetOnAxis(
                    ap=offt[:, i : i + 1], axis=0
                ),
            )
            X.append(Xi)

        for i in range(NI):
            Xi = X[i]
            d = work.tile([P, 3, 3, 2, C], bf16, tag=f"d{i}")
            nc.vector.tensor_tensor(
                out=d[:],
                in0=Xi[:],
                in1=Xi[:, 1:2, 1:2, :, :].to_broadcast([P, 3, 3, 2, C]),
                op=TT.subtract,
            )
            d2 = work.tile([P, 3, 3, 2, C], bf16, tag=f"d2{i}")
            nrm = work.tile([P, 3, 3, 2], bf16, tag=f"nrm{i}")
            for t in range(2):
                nc.scalar.square(
                    out=d2[:, :, :, t, :], in_=d[:, :, :, t, :]
                )
                with nc.allow_low_precision("norm2 bf16; loss gate 2e-2"):
                    nc.vector.tensor_reduce(
                        out=nrm[:, :, :, t : t + 1],
                        in_=d2[:, :, :, t, :],
                        axis=mybir.AxisListType.X,
                        op=TT.add,
                    )
            # sqrt(n + eps^2) == sqrt(n)+eps to ~1e-8 rel (exact at n=0)
            srt = work.tile([P, 3, 3, 2], f32, tag=f"srt{i}")
            nc.scalar.activation(
                out=srt[:],
                in_=nrm[:],
                func=mybir.ActivationFunctionType.Sqrt,
                bias=eps2[:, 0:1],
            )
            rr = work.tile([P, 3, 3, 2], f32, tag=f"rr{i}")
            nc.vector.reciprocal(out=rr[:], in_=srt[:])
            nc.sync.dma_start(out=out_rr[:, i, :, :, :], in_=rr[:])
            # rho = rk * (sqrt(nq)+eps);  |qh-kh| = rq * |dq - rho*dk|
            rho = work.tile([P, 3, 3, 1], f32, tag=f"rho{i}")
            nc.vector.tensor_tensor(
                out=rho[:],
                in0=rr[:, :, :, 1:2],
                in1=srt[:, :, :, 0:1],
                op=TT.mult,
            )
            kd = work.tile([P, 3, 3, C], bf16, tag=f"kd{i}")
            nc.vector.tensor_tensor(
                out=kd[:],
                in0=d[:, :, :, 1, :],
                in1=rho[:].to_broadcast([P, 3, 3, C]),
                op=TT.mult,
            )
            wt = work.tile([P, 3, 3, C], bf16, tag=f"wt{i}")
            nc.vector.tensor_tensor(
                out=wt[:],
                in0=d[:, :, :, 0, :],
                in1=kd[:],
                op=TT.subtract,
            )
            su = work.tile([P, 3, 3], bf16, tag=f"su{i}")
            with nc.allow_low_precision("|u| sums in bf16; loss gate 2e-2"):
                nc.vector.tensor_reduce(
                    out=su[:],
                    in_=wt[:],
                    axis=mybir.AxisListType.X,
                    op=TT.add,
                    apply_absolute_value=True,
                )
            nc.sync.dma_start(out=out_su[:, i, :, :], in_=su[:])

    _split_multi_waits(nc)
    return nc


def _stage_core(feat_q_b, feat_k_b):
    img = np.concatenate([feat_q_b, feat_k_b], axis=0)  # [128, H, W] f32
    img = img.astype(ml_dtypes.bfloat16)
    chl = np.ascontiguousarray(img.transpose(1, 2, 0))  # [H, W, 128] bf16
    pad = np.zeros((H + 2, W, P), dtype=ml_dtypes.bfloat16)
    pad[:H] = chl
    sv = np.lib.stride_tricks.as_strided(
        pad,
        (H, W, 3, P),
        (pad.strides[0], pad.strides[1], pad.strides[0], pad.strides[2]),
    )
    return np.ascontiguousarray(sv).reshape(HWPIX, FR)


def kernel(feat_q, feat_k, sample_ids, *, trace=False, trace_cores=None):
    global LAST_RESULTS
    feat_q = np.asarray(feat_q, dtype=np.float32)
    feat_k = np.asarray(feat_k, dtype=np.float32)
    ids = np.asarray(sample_ids).astype(np.int64)

    if "nc" not in _cache:
        _cache["nc"] = _build()
    nc = _cache["nc"]

    offs = (ids[:, 0] * W + ids[:, 1]).astype(np.int32)  # [256]
    offs = np.ascontiguousarray(offs.reshape(NI, P).T)   # [128, 2]

    in_maps = [
        {"fqk3": _stage_core(feat_q[b], feat_k[b]), "offs": offs}
        for b in range(N_CORES)
    ]
    results = run_bass_kernel_spmd(
        nc,
        in_maps,
        core_ids=list(range(N_CORES)),
        trace=trace,
        trace_cores=trace_cores,
    )
    LAST_RESULTS = results
    total = np.float64(0.0)
    for r in results.results:
        su = r["out_su"].astype(np.float64)
        rq = r["out_rr"][..., 0].astype(np.float64)
        total += (su * rq).sum()
    loss = total / (B * C * 8 * NUM_S)
    return np.asarray(loss, dtype=np.float32)


# revision 13
# speedup vs baseline: 1.0100x; 1.0100x over previous
"""CCPL contrastive-loss kernel for Trainium2 (8 NeuronCores).

The loss touches only 256 sampled 3x3 neighborhoods of the 512 MiB feat
tensors, so the kernel gathers exactly those windows and never streams the
full tensors.  Work is data-parallel over batch: core b gets feat_q[b] and
feat_k[b], staged to HBM as a 3-row-interleaved pixel-major bf16 tensor
  T[h*W + w] = [f(h+r, w, c) for r in 0..2 for c in 0..127]   (768 B/row)
(q's 64 channels then k's 64 channels per pixel; rows h+1, h+2 duplicated
into neighboring staged rows).  With this layout one sample's whole 3x3
window = 3 CONSECUTIVE staged rows = one contiguous 2304 B run.

Device dataflow per core:
  1. TWO indirect (SWDGE) DMAs -- one offset per partition, the
     hardware-verified form -- gather 128 samples each: partition p of
     instruction i receives the full window of sample i*128+p as
     [3(dw), 3(r), 2(q/k), 64(ch)] bf16 in its free dim.  sample_ids
     become a [128, 2] int32 SBUF offset table (h*512+w).
  2. Everything else is per-partition elementwise: center subtract
     (bf16, DVE 2x), square to f32 (ACT), channel-norm reduce (DVE),
     sqrt(n + eps^2) via ACT bias (== sqrt(n)+eps to 1e-8 rel), and the
     normalized difference via the exact factorization
         |qh - kh| = rq * |dq - (rk*(sqrt(nq)+eps)) * dk|
     which saves one full-width multiply pass.  One q-k subtract runs on
     GPSIMD to offload DVE.
  3. Partials [128, 2, 3, 3] DMA out per block; host sums and divides.

The center slot produces exactly zero contribution (its window difference
is identically 0 and rho=1 there), so no masking is needed.
"""

import os
import sys
from contextlib import ExitStack

import ml_dtypes
import numpy as np

sys.path.insert(0, "/opt/trn_rl_repo")

import concourse.bass as bass
import concourse.tile as tile
from concourse import mybir
from concourse.bass_utils import run_bass_kernel_spmd


def _install_ntff_hook():
    """Provide antenv.axon_hooks when the agent image lacks it."""
    try:
        from antenv.axon_hooks import get_axon_ntff_profile_hook  # noqa: F401

        return
    except ImportError:
        pass
    import types

    hook = None
    try:
        from trn_agent_boot.trn_boot import _ntff_profile_via_ctypes

        so = "/opt/axon/libaxon_pjrt.so"
        if os.path.exists(so):
            hook = _ntff_profile_via_ctypes(so)
    except Exception:
        hook = None
    mod = types.ModuleType("antenv.axon_hooks")
    _state = {"hook": hook}
    mod.get_axon_ntff_profile_hook = lambda: _state["hook"]
    mod.set_axon_ntff_profile_hook = lambda h: _state.update(hook=h)
    import antenv

    sys.modules["antenv.axon_hooks"] = mod
    antenv.axon_hooks = mod


_install_ntff_hook()

B, C, H, W = 8, 64, 512, 512
NUM_S = 256
EPS = 1e-7
P = 128
HWPIX = H * W
NI = NUM_S // P     # 2 gather instructions, 128 samples each
FR = 3 * 2 * C      # 384 elements per staged row
N_CORES = 8

_cache: dict = {}
LAST_RESULTS = None  # BassKernelResults of the most recent run (for test.py)


def _split_multi_waits(nc):
    """Walrus build here embeds at most ONE sync wait per instruction."""
    from concourse import mybir as _mybir

    for f in nc.m.functions:
        for blk in f.blocks:
            insts = blk.instructions
            i = 0
            while i < len(insts):
                inst = insts[i]
                si = inst.sync_info
                if si is not None and si.on_wait and len(si.on_wait) > 1:
                    waits = list(si.on_wait)
                    si.on_wait = waits[-1:]
                    for j, w in enumerate(waits[:-1]):
                        nop = _mybir.InstNoOp(
                            name=nc.get_next_instruction_name(),
                            ins=[],
                            outs=[],
                            engine=inst.engine,
                            sync_info=_mybir.SyncInfo(on_wait=[w], on_update=[]),
                        )
                        insts.insert(i + j, nop)
                    i += len(waits) - 1
                i += 1


def _build():
    f32 = mybir.dt.float32
    bf16 = mybir.dt.bfloat16
    i32 = mybir.dt.int32
    TT = mybir.AluOpType
    nc = bass.Bass()
    fqk3 = nc.dram_tensor("fqk3", [HWPIX, FR], bf16, kind="ExternalInput")
    offs = nc.dram_tensor("offs", [P, NI], i32, kind="ExternalInput")
    out_su = nc.dram_tensor("out_su", [P, NI, 3, 3], bf16, kind="ExternalOutput")
    out_rr = nc.dram_tensor("out_rr", [P, NI, 3, 3, 2], f32, kind="ExternalOutput")

    with tile.TileContext(nc) as tc, ExitStack() as ctx:
        sb = ctx.enter_context(tc.tile_pool(name="sb", bufs=1))
        work = ctx.enter_context(tc.tile_pool(name="work", bufs=1))

        offt = sb.tile([P, NI], i32)
        nc.sync.dma_start(out=offt[:], in_=offs[:])
        eps2 = sb.tile([P, 1], f32)
        nc.vector.memset(eps2[:], EPS * EPS)

        X = []
        for i in range(NI):
            Xi = work.tile([P, 3, 3, 2, C], bf16, tag=f"x{i}")
            nc.gpsimd.indirect_dma_start(
                out=Xi[:].rearrange("p a b t c -> p (a b t c)"),
                out_offset=None,
                in_=fqk3[:],
                in_offset=bass.IndirectOffsetOnAxis(
                    ap=offt[:, i : i + 1], axis=0
                ),
            )
            X.append(Xi)

        for i in range(NI):
            Xi = X[i]
            d = work.tile([P, 3, 3, 2, C], bf16, tag=f"d{i}")
            nc.vector.tensor_tensor(
                out=d[:],
                in0=Xi[:],
                in1=Xi[:, 1:2, 1:2, :, :].to_broadcast([P, 3, 3, 2, C]),
                op=TT.subtract,
            )
            d2 = work.tile([P, 3, 3, 2, C], bf16, tag=f"d2{i}")
            nrm = work.tile([P, 3, 3, 2], bf16, tag=f"nrm{i}")
            for t in range(2):
                if i == 0 and t == 0:
                    # DVE bf16 2x multiply fills the stall while gather 1's
                    # data is still in flight; also skips the ACT->DVE hop.
                    nc.vector.tensor_tensor(
                        out=d2[:, :, :, 0, :],
                        in0=d[:, :, :, 0, :],
                        in1=d[:, :, :, 0, :],
                        op=TT.mult,
                    )
                else:
                    nc.scalar.square(
                        out=d2[:, :, :, t, :], in_=d[:, :, :, t, :]
                    )
                with nc.allow_low_precision("norm2 bf16; loss gate 2e-2"):
                    nc.vector.tensor_reduce(
                        out=nrm[:, :, :, t : t + 1],
                        in_=d2[:, :, :, t, :],
                        axis=mybir.AxisListType.X,
                        op=TT.add,
                    )
            # sqrt(n + eps^2) == sqrt(n)+eps to ~1e-8 rel (exact at n=0)
            srt = work.tile([P, 3, 3, 2], f32, tag=f"srt{i}")
            nc.scalar.activation(
                out=srt[:],
                in_=nrm[:],
                func=mybir.ActivationFunctionType.Sqrt,
                bias=eps2[:, 0:1],
            )
            rr = work.tile([P, 3, 3, 2], f32, tag=f"rr{i}")
            nc.vector.reciprocal(out=rr[:], in_=srt[:])
            nc.sync.dma_start(out=out_rr[:, i, :, :, :], in_=rr[:])
            # rho = rk * (sqrt(nq)+eps);  |qh-kh| = rq * |dq - rho*dk|
            rho = work.tile([P, 3, 3, 1], f32, tag=f"rho{i}")
            nc.vector.tensor_tensor(
                out=rho[:],
                in0=rr[:, :, :, 1:2],
                in1=srt[:, :, :, 0:1],
                op=TT.mult,
            )
            kd = work.tile([P, 3, 3, C], bf16, tag=f"kd{i}")
            nc.vector.tensor_tensor(
                out=kd[:],
                in0=d[:, :, :, 1, :],
                in1=rho[:].to_broadcast([P, 3, 3, C]),
                op=TT.mult,
            )
            wt = work.tile([P, 3, 3, C], bf16, tag=f"wt{i}")
            nc.vector.tensor_tensor(
                out=wt[:],
                in0=d[:, :, :, 0, :],
                in1=kd[:],
                op=TT.subtract,
            )
            su = work.tile([P, 3, 3], bf16, tag=f"su{i}")
            with nc.allow_low_precision("|u| sums in bf16; loss gate 2e-2"):
                nc.vector.tensor_reduce(
                    out=su[:],
                    in_=wt[:],
                    axis=mybir.AxisListType.X,
                    op=TT.add,
                    apply_absolute_value=True,
                )
            nc.sync.dma_start(out=out_su[:, i, :, :], in_=su[:])

    _split_multi_waits(nc)
    return nc


def _stage_core(feat_q_b, feat_k_b):
    img = np.concatenate([feat_q_b, feat_k_b], axis=0)  # [128, H, W] f32
    img = img.astype(ml_dtypes.bfloat16)
    chl = np.ascontiguousarray(img.transpose(1, 2, 0))  # [H, W, 128] bf16
    pad = np.zeros((H + 2, W, P), dtype=ml_dtypes.bfloat16)
    pad[:H] = chl
    sv = np.lib.stride_tricks.as_strided(
        pad,
        (H, W, 3, P),
        (pad.strides[0], pad.strides[1], pad.strides[0], pad.strides[2]),
    )
    return np.ascontiguousarray(sv).reshape(HWPIX, FR)


def kernel(feat_q, feat_k, sample_ids, *, trace=False, trace_cores=None):
    global LAST_RESULTS
    feat_q = np.asarray(feat_q, dtype=np.float32)
    feat_k = np.asarray(feat_k, dtype=np.float32)
    ids = np.asarray(sample_ids).astype(np.int64)

    if "nc" not in _cache:
        _cache["nc"] = _build()
    nc = _cache["nc"]

    offs = (ids[:, 0] * W + ids[:, 1]).astype(np.int32)  # [256]
    offs = np.ascontiguousarray(offs.reshape(NI, P).T)   # [128, 2]

    in_maps = [
        {"fqk3": _stage_core(feat_q[b], feat_k[b]), "offs": offs}
        for b in range(N_CORES)
    ]
    results = run_bass_kernel_spmd(
        nc,
        in_maps,
        core_ids=list(range(N_CORES)),
        trace=trace,
        trace_cores=trace_cores,
    )
    LAST_RESULTS = results
    total = np.float64(0.0)
    for r in results.results:
        su = r["out_su"].astype(np.float64)
        rq = r["out_rr"][..., 0].astype(np.float64)
        total += (su * rq).sum()
    loss = total / (B * C * 8 * NUM_S)
    return np.asarray(loss, dtype=np.float32)
